# revision 6
# baseline (speedup 1.0000x reference)
"""HPSS (harmonic/percussive source separation) Trainium2 kernel, v4.

Input S [2,2,1025,1024] f32. Per (b,c) plane: harm = median-31 along W
(zero-padded), perc = median-31 along H; softmask with power=2, margin=1;
returns (S*mask_h, S*mask_p).

Sharding: 8 cores = 4 planes x 2 W-halves. Each core computes, for its
plane/half: perc medians for its 512 columns over all 1025 rows, harm
medians + mask_h outputs for rows 0..1023 x its 512 columns. Row 1024
(one row per plane) is finished on the host. mask_p = 1 - mask_h exactly
(margin=1), so out_p = S - out_h is computed on the host; the device only
emits OH.

v4 key change vs v3: band-compact Gil-Werman. Prefix order-stat level m
is only ever read at block positions [m-1, 14+m] (by the next level's
shift-max and by exactly one merge layer), and suffix level l only at
[16-l, 31-l]. So every level >= 2 is stored as a compact width-16 tile
per 31-block and its min-scan covers 16/31 of the data. In compact slot
coordinates the chain and merge reads align at the SAME slot for every
level, so no shifts are needed: level m reads level m-1 at slot s and
x at block position m-1+s. This cuts DVE scan work ~45% and shrinks all
shift-max traffic by 16/31.

Engine split (tuned against the scheduler sim): scans on DVE; shift-max
via the exact identity max(p,x) = x + relu(p-x) on Pool+Act (fp32
intermediates); merge max/min native fp16 tensor_tensor on DVE, with a
tunable subset emulated on Pool+Act.
"""
import sys

import numpy as np

sys.path.insert(0, "/opt/trn_rl_repo")

P = 128
K = 31
LEV = 16
BW = 16             # compact band width per level
GUARD = 2.0
MASKV = 32768.0
HALF = 15
NB_H = 18           # harm blocks per 558-col local strip
NH = NB_H * K       # 558
NB_P = 35           # perc blocks (1085 padded H)
NP = NB_P * K       # 1085
QB = 2              # row-groups per harm batch
NHB = QB * NH       # 1116 free elems per harm batch
NPB = NP            # 1085 free elems per perc batch

_PROGRAM = None
_SIM_NS = None

# levels (2..16) whose shift-max runs as add/relu on Pool+Act rather than
# native fp16 max on DVE
EMU_PRE = set(range(2, 17))
EMU_SUF = set(range(2, 17))
# merge layers (1..15) whose tm = max(suf,pre) is emulated on Pool+Act
EMU_MRG = set()


def _build_program():
    from contextlib import ExitStack

    import concourse.mybir as mybir
    import concourse.tile as tile
    from concourse import bacc

    f32 = mybir.dt.float32
    f16 = mybir.dt.float16
    MIN = mybir.AluOpType.min
    MAX = mybir.AluOpType.max
    ADD = mybir.AluOpType.add
    MULT = mybir.AluOpType.mult

    from bass_rust import ActivationFunctionType as AF

    nc = bacc.Bacc("TRN2", target_bir_lowering=False, debug=False)
    XH = nc.declare_dram_parameter("XH", [1024, NH], f16, isOutput=False)
    XP = nc.declare_dram_parameter("XP", [512, NP], f16, isOutput=False)
    ID = nc.declare_dram_parameter("ID", [P, P], f16, isOutput=False)
    PM1024 = nc.declare_dram_parameter("PM1024", [512, 1], f16, isOutput=True)
    OH = nc.declare_dram_parameter("OH", [1024, 512], f16, isOutput=True)

    def median_banded(pools, x2, N, ridx):
        """x2: [P, N] fp16 AP (N = nb*K). Returns cmin tile [P, N] with
        cmin[31*i + o] = median of x2[31*i+o .. +30] for i <= nb-2."""
        setid = ridx % 2
        levpool, cmpool, maskF, maskC = pools
        nb = N // K
        ic = nb - 1
        CN = nb * BW
        x3 = x2.rearrange("p (b k) -> p b k", k=K)
        mF = maskF[:, 0:N]
        mC = maskC[:, 0:CN]
        q = nb // 4
        hv = [(0, q), (q, 2 * q), (2 * q, 3 * q), (3 * q, nb)]

        # negated input for the add/relu max-emulation on Pool+Act
        nx = levpool.tile([P, N], f16, tag=f"nx_{setid}", name="nx")
        nc.scalar.activation(nx[:], x2, AF.Copy, bias=0.0, scale=-1.0)
        nx3 = nx[:].rearrange("p (b k) -> p b k", k=K)

        # fp32 scratch for emulation (compact width)
        dp = levpool.tile([P, CN], f32, tag=f"dp_{setid}", name="dp")
        rp = levpool.tile([P, CN], f32, tag=f"rp_{setid}", name="rp")
        ds = levpool.tile([P, CN], f32, tag=f"ds_{setid}", name="ds")
        rs = levpool.tile([P, CN], f32, tag=f"rs_{setid}", name="rs")

        # level-1 chains: full width, scan x directly
        pre1 = levpool.tile([P, N], f16, tag=f"pre1_{ridx % 3}", name="pre1")
        suf1 = levpool.tile([P, N], f16, tag=f"suf1_{setid}", name="suf1")
        for c0, c1 in hv:
            nc.vector.tensor_tensor_scan(pre1[:, c0 * K:c1 * K],
                                         mF[:, c0 * K:c1 * K],
                                         x2[:, c0 * K:c1 * K],
                                         GUARD, op0=ADD, op1=MIN)
        pre13 = pre1[:].rearrange("p (b k) -> p b k", k=K)

        # compact prefix levels 2..16
        preC = {1: None}
        tpre = levpool.tile([P, CN], f16, tag=f"tpre_{setid}", name="tpre")
        for m in range(2, LEV + 1):
            pc = levpool.tile([P, CN], f16,
                              tag=f"preC{m}_{ridx % 3 if m < 5 else setid}",
                              name=f"preC{m}")
            src3 = (pre13[:, :, 0:BW] if m == 2 else
                    preC[m - 1][:].rearrange("p (b s) -> p b s", s=BW))
            t3 = tpre[:].rearrange("p (b s) -> p b s", s=BW)
            d3 = dp[:].rearrange("p (b s) -> p b s", s=BW)
            r3 = rp[:].rearrange("p (b s) -> p b s", s=BW)
            xb = x3[:, :, m - 1:m - 1 + BW]
            nxb = nx3[:, :, m - 1:m - 1 + BW]
            for c0, c1 in hv:
                if m in EMU_PRE:
                    nc.gpsimd.tensor_tensor(d3[:, c0:c1], src3[:, c0:c1],
                                            nxb[:, c0:c1], op=ADD)
                    nc.scalar.activation(r3[:, c0:c1], d3[:, c0:c1], AF.Relu)
                    nc.gpsimd.tensor_tensor(t3[:, c0:c1], xb[:, c0:c1],
                                            r3[:, c0:c1], op=ADD)
                else:
                    nc.vector.tensor_tensor(t3[:, c0:c1], src3[:, c0:c1],
                                            xb[:, c0:c1], op=MAX)
                nc.vector.tensor_tensor_scan(pc[:, c0 * BW:c1 * BW],
                                             mC[:, c0 * BW:c1 * BW],
                                             tpre[:, c0 * BW:c1 * BW],
                                             GUARD, op0=ADD, op1=MIN)
            preC[m] = pc

        # output accumulator; slots [0:16] start at GUARD, [16:31] from the
        # layer-0 copy of prefix level 16 (block i+1 slots 0..14)
        cmin = cmpool.tile([P, N], f16, tag=f"cm_{setid}", name="cm")
        cm3 = cmin[:].rearrange("p (b k) -> p b k", k=K)
        nc.scalar.activation(cm3[:, :, 0:BW], x3[:, :, 0:BW], AF.Copy,
                             bias=GUARD, scale=0.0)
        pc163 = preC[16][:].rearrange("p (b s) -> p b s", s=BW)
        nc.scalar.copy(cm3[:, 0:ic, 16:31], pc163[:, 1:1 + ic, 0:15])

        # suffix chain levels 1..16 (compact from 2), merge as levels arrive
        for c0, c1 in hv:
            nc.vector.tensor_tensor_scan(
                suf1[:, c0 * K:c1 * K][:, ::-1], mF[:, c0 * K:c1 * K],
                x2[:, c0 * K:c1 * K][:, ::-1], GUARD, op0=ADD, op1=MIN)
        suf13 = suf1[:].rearrange("p (b k) -> p b k", k=K)

        sufq = [levpool.tile([P, CN], f16, tag=f"sufC{i}_{setid}",
                             name=f"sufC{i}") for i in range(2)]
        tsuf = levpool.tile([P, CN], f16, tag=f"tsuf_{setid}", name="tsuf")
        tm = levpool.tile([P, CN], f16, tag=f"tm_{setid}", name="tm")
        dm = levpool.tile([P, CN], f32, tag=f"dm_{setid}", name="dm")
        rm = levpool.tile([P, CN], f32, tag=f"rm_{setid}", name="rm")
        npb = levpool.tile([P, CN], f16, tag=f"npb_{setid}", name="npb")
        tm3 = tm[:].rearrange("p (b s) -> p b s", s=BW)

        for lay in range(1, LEV + 1):
            # compute suffix level lay (compact for lay >= 2)
            if lay == 1:
                a3 = suf13[:, 0:ic, 15:31]
            else:
                cur = sufq[lay % 2]
                src3 = (suf13[:, :, 15:31] if lay == 2 else
                        sufq[(lay - 1) % 2][:].rearrange(
                            "p (b s) -> p b s", s=BW))
                t3 = tsuf[:].rearrange("p (b s) -> p b s", s=BW)
                d3 = ds[:].rearrange("p (b s) -> p b s", s=BW)
                r3 = rs[:].rearrange("p (b s) -> p b s", s=BW)
                xb = x3[:, :, 16 - lay:32 - lay]
                nxb = nx3[:, :, 16 - lay:32 - lay]
                for c0, c1 in hv:
                    if lay in EMU_SUF:
                        nc.gpsimd.tensor_tensor(d3[:, c0:c1], src3[:, c0:c1],
                                                nxb[:, c0:c1], op=ADD)
                        nc.scalar.activation(r3[:, c0:c1], d3[:, c0:c1],
                                             AF.Relu)
                        nc.gpsimd.tensor_tensor(t3[:, c0:c1], xb[:, c0:c1],
                                                r3[:, c0:c1], op=ADD)
                    else:
                        nc.vector.tensor_tensor(t3[:, c0:c1], src3[:, c0:c1],
                                                xb[:, c0:c1], op=MAX)
                    nc.vector.tensor_tensor_scan(
                        cur[:, c0 * BW:c1 * BW][:, ::-1],
                        mC[:, c0 * BW:c1 * BW],
                        tsuf[:, c0 * BW:c1 * BW][:, ::-1],
                        GUARD, op0=ADD, op1=MIN)
                a3 = cur[:].rearrange("p (b s) -> p b s", s=BW)[:, 0:ic, :]

            # merge layer lay
            cs = cm3[:, 0:ic, 16 - lay:32 - lay]
            if lay == LEV:
                nc.vector.tensor_tensor(cs, cs, a3, op=MIN)
            else:
                m = LEV - lay
                b3 = (pre13[:, 1:1 + ic, 0:BW] if m == 1 else
                      preC[m][:].rearrange(
                          "p (b s) -> p b s", s=BW)[:, 1:1 + ic, :])
                ts = tm3[:, 0:ic, :]
                if lay in EMU_MRG:
                    nb3 = npb[:].rearrange(
                        "p (b s) -> p b s", s=BW)[:, 0:ic, :]
                    dm3 = dm[:].rearrange(
                        "p (b s) -> p b s", s=BW)[:, 0:ic, :]
                    rm3 = rm[:].rearrange(
                        "p (b s) -> p b s", s=BW)[:, 0:ic, :]
                    nc.scalar.activation(nb3, b3, AF.Copy, bias=0.0,
                                         scale=-1.0)
                    nc.gpsimd.tensor_tensor(dm3, a3, nb3, op=ADD)
                    nc.scalar.activation(rm3, dm3, AF.Relu)
                    nc.gpsimd.tensor_tensor(ts, b3, rm3, op=ADD)
                else:
                    nc.vector.tensor_tensor(ts, a3, b3, op=MAX)
                nc.vector.tensor_tensor(cs, cs, ts, op=MIN)
        return cmin

    with tile.TileContext(nc) as tc:
        with ExitStack() as ctx:
            cpool = ctx.enter_context(tc.tile_pool(name="const", bufs=1))
            inpool = ctx.enter_context(tc.tile_pool(name="in", bufs=2))
            levpool = ctx.enter_context(tc.tile_pool(name="lev", bufs=1))
            cmpool = ctx.enter_context(tc.tile_pool(name="cm", bufs=2))
            pcpool = ctx.enter_context(tc.tile_pool(name="pc", bufs=1))
            sfpool = ctx.enter_context(tc.tile_pool(name="sf", bufs=2))
            ppool = ctx.enter_context(tc.tile_pool(name="ps", bufs=2,
                                                   space="PSUM"))

            NBMAX = max(NHB, NPB)
            CNMAX = (NBMAX // K) * BW
            maskF = cpool.tile([P, NBMAX], f16)
            nc.vector.memset(maskF[:], 0.0)
            mF3 = maskF[:].rearrange("p (b k) -> p b k", k=K)
            nc.vector.memset(mF3[:, :, 0:1], MASKV)
            maskC = cpool.tile([P, CNMAX], f16)
            nc.vector.memset(maskC[:], 0.0)
            mC3 = maskC[:].rearrange("p (b s) -> p b s", s=BW)
            nc.vector.memset(mC3[:, :, 0:1], MASKV)
            ident = cpool.tile([P, P], f16)
            nc.sync.dma_start(ident[:], ID[:])

            pools = (levpool, cmpool, maskF, maskC)

            # ---- phase A: perc (median along H), 4 col-group batches
            pcm = []
            for a in range(4):
                xp = inpool.tile([P, NP], f16, tag="xp", name="xp")
                nc.sync.dma_start(
                    xp[:], XP[:].rearrange("(a p) n -> a p n", p=P)[a])
                cmin = median_banded(pools, xp[:], NP, a)
                pc = pcpool.tile([P, NP], f16, tag=f"pcm{a}", name=f"pcm{a}")
                nc.scalar.copy(pc[:], cmin[:])
                pcm.append(pc)
                nc.sync.dma_start(
                    PM1024[:].rearrange("(a p) o -> a p o", p=P)[a],
                    pc[:, 1024:1025])

            # ---- phase B: harm (median along W) + softmask, QB rows per batch
            for bi in range(4):
                xh = inpool.tile([P, QB, NH], f16, tag="xh", name="xh")
                nc.sync.dma_start(
                    xh[:], XH[:].rearrange("(b q p) n -> b p q n", p=P, q=QB)[bi])
                cmin = median_banded(pools,
                                     xh[:].rearrange("p q n -> p (q n)"),
                                     NHB, 4 + bi)

                # perc slices for rows of this batch, via PE transpose
                percT = sfpool.tile([P, QB, 512], f16, tag="percT", name="percT")
                for qq in range(QB):
                    r0 = bi * QB * P + qq * P
                    for cg in range(4):
                        ps = ppool.tile([P, P], f16, tag="ps", name="ps")
                        nc.tensor.transpose(
                            ps[:], pcm[cg][:, r0:r0 + P], ident[:])
                        nc.scalar.copy(percT[:, qq, cg * P:(cg + 1) * P], ps[:])

                # softmask: oh = S*h^2/(h^2+p^2); out_p = S - oh on host
                cm4 = cmin[:].rearrange("p (q n) -> p q n", n=NH)
                h = cm4[:, :, 0:512]
                s_in = xh[:, :, HALF:HALF + 512]
                h2 = sfpool.tile([P, QB, 512], f16, tag="h2", name="h2")
                den = sfpool.tile([P, QB, 512], f16, tag="den", name="den")
                rden = sfpool.tile([P, QB, 512], f16, tag="rden", name="rden")
                # scale 64 inside the squares keeps (64h)^2 out of the
                # fp16 subnormal range (Act flushes subnormals to 0); the
                # 4096x factor cancels in h2*rden since den scales too.
                nc.scalar.activation(h2[:], h, AF.Square, scale=64.0)
                nc.scalar.activation(percT[:], percT[:], AF.Square, scale=64.0)
                nc.gpsimd.tensor_tensor(den[:], h2[:], percT[:], op=ADD)
                with nc.allow_low_precision(reason="den >= 1.1e-4 on data"):
                    nc.vector.reciprocal(rden[:], den[:])
                    nc.gpsimd.tensor_tensor(h2[:], h2[:], rden[:], op=MULT)
                    nc.gpsimd.tensor_tensor(h2[:], h2[:], s_in, op=MULT)
                oh_d = OH[:].rearrange("(b q p) n -> b p q n", p=P, q=QB)[bi]
                nc.sync.dma_start(oh_d, h2[:])

        ret = tc.schedule_and_allocate()
        global _SIM_NS
        try:
            _SIM_NS = ret[1].time
        except Exception:
            _SIM_NS = None

    nc.finalize()
    return nc


def _get_program():
    global _PROGRAM
    if _PROGRAM is None:
        _PROGRAM = _build_program()
    return _PROGRAM


def _host_prep(S):
    """Returns in_maps (8 dicts) of fp16 arrays."""
    ident = np.eye(P, dtype=np.float16)
    S16 = S.astype(np.float16)
    in_maps = []
    for c in range(8):
        pl, h = c >> 1, c & 1
        b, ch = pl >> 1, pl & 1
        Sp = S16[b, ch]
        xh = np.zeros((1024, NH), np.float16)
        lo = 512 * h - HALF
        s0, s1 = max(0, lo), min(1024, lo + NH)
        xh[:, s0 - lo:s1 - lo] = Sp[0:1024, s0:s1]
        xp = np.zeros((512, NP), np.float16)
        xp[:, HALF:HALF + 1025] = Sp[:, 512 * h:512 * h + 512].T
        in_maps.append({"XH": xh, "XP": xp, "ID": ident})
    return in_maps


def _median31_rows(rows):
    """Exact median-31 along last axis with zero pad; rows [R, 1024]."""
    R, W = rows.shape
    p = np.pad(rows, ((0, 0), (HALF, HALF)))
    win = np.lib.stride_tricks.sliding_window_view(p, K, axis=1)
    return np.median(win, axis=2).astype(np.float32)


def kernel(S):
    from concourse.bass_utils import run_bass_kernel_spmd

    S = np.asarray(S, np.float32)
    nc = _get_program()
    in_maps = _host_prep(S)
    res = run_bass_kernel_spmd(nc, in_maps, list(range(8)))

    out_h = np.empty_like(S)
    perc_1024 = np.empty((2, 2, 1024), np.float32)
    for c in range(8):
        pl, h = c >> 1, c & 1
        b, ch = pl >> 1, pl & 1
        r = res.results[c]
        out_h[b, ch, 0:1024, 512 * h:512 * h + 512] = r["OH"].astype(np.float32)
        perc_1024[b, ch, 512 * h:512 * h + 512] = \
            r["PM1024"][:, 0].astype(np.float32)
    # host: row 1024 of each plane (harm median of 4 rows + device perc row)
    rows = S[:, :, 1024, :].reshape(4, 1024)
    harm_1024 = _median31_rows(rows).reshape(2, 2, 1024)
    h2 = harm_1024 * harm_1024
    p2 = perc_1024 * perc_1024
    out_h[:, :, 1024, :] = S[:, :, 1024, :] * h2 / (h2 + p2)
    # mask_p = 1 - mask_h exactly (margin=1) => out_p = S - out_h
    out_p = S - out_h
    return out_h, out_p


# revision 9
# speedup vs baseline: 1.0969x; 1.0969x over previous
"""HPSS (harmonic/percussive source separation) Trainium2 kernel, v5.

Input S [2,2,1025,1024] f32. Per (b,c) plane: harm = median-31 along W
(zero-padded), perc = median-31 along H; softmask with power=2, margin=1;
returns (S*mask_h, S*mask_p).

Sharding: 8 cores = 4 planes x 2 W-halves. Each core computes perc
medians for its 512 columns over all 1025 rows, harm medians + mask_h
outputs for rows 0..1023 x its 512 columns. Row 1024 is finished on the
host. mask_p = 1 - mask_h exactly (margin=1), so out_p = S - out_h on
the host; the device only emits OH.

Band-compact Gil-Werman (v4): prefix order-stat level m is only ever
read at block positions [m-1, 14+m], suffix level l at [16-l, 31-l], so
levels >= 2 are width-16 compact tiles and their min-scans cover 16/31
of the data. In compact slot coordinates every read aligns at the SAME
slot across levels.

v5 on top of v4:
- prefix+suffix fused per level: one [P, 2, CN] plane-pair tile per
  level; the shift-max emulation (max(p,x) = x + relu(p-x)) runs as
  single double-width ops using custom 4-D access patterns over x/nx
  (two bands at constant stride 17-2m apart), halving per-op fixed
  costs on Pool/Act.
- per-level emulation flavor: "pa" = Pool add / Act relu / Pool add
  (fp32 intermediates, bit-exact), "pp" = Pool-only with relu as
  tensor_scalar max(d,0) (fp32, bit-exact, no cross-engine hops),
  "nat" = native fp16 max on DVE.
- merge layers interleaved pairwise into the level loop: layer lay
  needs (suf_lay, pre_{16-lay}), ready at step max(lay,16-lay); levels
  2..8 are kept live, 9..16 ping-pong, halving SBUF.
"""
import sys

import numpy as np

sys.path.insert(0, "/opt/trn_rl_repo")

P = 128
K = 31
LEV = 16
BW = 16
GUARD = 2.0
MASKV = 32768.0
HALF = 15
NB_H = 18
NH = NB_H * K       # 558
NB_P = 35
NP = NB_P * K       # 1085
QB = 2
NHB = QB * NH       # 1116
NPB = NP            # 1085

_PROGRAM = None
_SIM_NS = None

# emulation flavor per level 2..16: "pa", "pp", or "nat"
FLAVOR = {m: ("pa" if m % 2 == 0 else "pp") for m in range(2, 17)}
# merge layers 1..15 with emulated tm-max (Act negate + Pool) — default
# native DVE
EMU_MRG = set()


def _build_program():
    from contextlib import ExitStack

    import concourse.mybir as mybir
    import concourse.tile as tile
    from concourse import bacc
    from concourse.ap import AP

    f32 = mybir.dt.float32
    f16 = mybir.dt.float16
    MIN = mybir.AluOpType.min
    MAX = mybir.AluOpType.max
    ADD = mybir.AluOpType.add
    MULT = mybir.AluOpType.mult

    from bass_rust import ActivationFunctionType as AF

    nc = bacc.Bacc("TRN2", target_bir_lowering=False, debug=False)
    XH = nc.declare_dram_parameter("XH", [1024, NH], f16, isOutput=False)
    XP = nc.declare_dram_parameter("XP", [512, NP], f16, isOutput=False)
    ID = nc.declare_dram_parameter("ID", [P, P], f16, isOutput=False)
    PM1024 = nc.declare_dram_parameter("PM1024", [512, 1], f16, isOutput=True)
    OH = nc.declare_dram_parameter("OH", [1024, 512], f16, isOutput=True)

    def dual_band(t, off0, off1, c0, c1, w=BW):
        """[P, 2, c1-c0, w] AP over (flat-viewed) tile t: plane 0 at
        block position off0, plane 1 at off1, blocks c0..c1."""
        a = t[:]
        part = list(list(a.ap)[0])
        return AP(a.tensor, a.offset + off0 + c0 * K,
                  [part, [off1 - off0, 2], [K, c1 - c0], [1, w]])

    def median_banded(pools, xt, x2, N, ridx):
        """x2: [P, N] fp16 AP over tile xt (N = nb*K). Returns cmin tile
        [P, N], cmin[31*i+o] = median of x2[31*i+o .. +30], i <= nb-2."""
        setid = ridx % 2
        levpool, cmpool, maskF, maskC = pools
        nb = N // K
        ic = nb - 1
        CN = nb * BW
        x3 = x2.rearrange("p (b k) -> p b k", k=K)
        mF = maskF[:, 0:N]
        mC = maskC[:, 0:CN]
        hv = [(0, nb // 2), (nb // 2, nb)]

        nx = levpool.tile([P, N], f16, tag=f"nx_{setid}", name="nx")
        nc.scalar.activation(nx[:], x2, AF.Copy, bias=0.0, scale=-1.0)

        # level 1 (both planes in one tile): full-width scans of x
        ps1 = levpool.tile([P, 2, N], f16, tag=f"ps1_{setid}", name="ps1")
        for c0, c1 in hv:
            nc.vector.tensor_tensor_scan(ps1[:, 0, c0 * K:c1 * K],
                                         mF[:, c0 * K:c1 * K],
                                         x2[:, c0 * K:c1 * K],
                                         GUARD, op0=ADD, op1=MIN)
            nc.vector.tensor_tensor_scan(ps1[:, 1, c0 * K:c1 * K][:, ::-1],
                                         mF[:, c0 * K:c1 * K],
                                         x2[:, c0 * K:c1 * K][:, ::-1],
                                         GUARD, op0=ADD, op1=MIN)
        pre13 = ps1[:, 0, :].rearrange("p (b k) -> p b k", k=K)
        suf13 = ps1[:, 1, :].rearrange("p (b k) -> p b k", k=K)

        # output accumulator, GUARD-filled; merge layers min into it
        cmin = cmpool.tile([P, N], f16, tag=f"cm_{setid}", name="cm")
        cm3 = cmin[:].rearrange("p (b k) -> p b k", k=K)
        nc.scalar.activation(cmin[:], x2, AF.Copy, bias=GUARD, scale=0.0)

        # scratch for fused emu
        dsc = levpool.tile([P, 2, CN], f32, tag=f"dsc_{setid}", name="dsc")
        rsc = levpool.tile([P, 2, CN], f32, tag=f"rsc_{setid}", name="rsc")
        tsc = levpool.tile([P, 2, CN], f16, tag=f"tsc_{setid}", name="tsc")
        tm = levpool.tile([P, CN], f16, tag=f"tm_{setid}", name="tm")
        npb = levpool.tile([P, CN], f16, tag=f"npb_{setid}", name="npb")

        pcq = {}

        def plane(m, pl):
            return pcq[m][:, pl, :].rearrange("p (b s) -> p b s", s=BW)

        def merge_layer(lay):
            cs = cm3[:, 0:ic, 16 - lay:32 - lay]
            if lay == 1:
                a3 = suf13[:, 0:ic, 15:31]
            else:
                a3 = plane(lay, 1)[:, 0:ic, :]
            if lay == LEV:
                nc.vector.tensor_tensor(cs, cs, a3, op=MIN)
                return
            m16 = LEV - lay
            if m16 == 1:
                b3 = pre13[:, 1:1 + ic, 0:BW]
            else:
                b3 = plane(m16, 0)[:, 1:1 + ic, :]
            ts = tm[:].rearrange("p (b s) -> p b s", s=BW)[:, 0:ic, :]
            if lay in EMU_MRG:
                nb3 = npb[:].rearrange("p (b s) -> p b s", s=BW)[:, 0:ic, :]
                nc.scalar.activation(nb3, b3, AF.Copy, bias=0.0, scale=-1.0)
                d3 = dsc[:, 0, :].rearrange(
                    "p (b s) -> p b s", s=BW)[:, 0:ic, :]
                nc.gpsimd.tensor_tensor(d3, a3, nb3, op=ADD)
                nc.gpsimd.tensor_scalar(d3, d3, 0.0, None, op0=MAX)
                nc.gpsimd.tensor_tensor(ts, b3, d3, op=ADD)
            else:
                nc.vector.tensor_tensor(ts, a3, b3, op=MAX)
            nc.vector.tensor_tensor(cs, cs, ts, op=MIN)

        for m in range(2, LEV + 1):
            if m <= 8:
                pcq[m] = levpool.tile([P, 2, CN], f16,
                                      tag=f"pcq{m}_{setid}", name=f"pcq{m}")
            else:
                pcq[m] = levpool.tile([P, 2, CN], f16,
                                      tag=f"pcqh{m % 2}_{setid}",
                                      name=f"pcq{m}")
            fl = FLAVOR[m]
            t4 = tsc[:].rearrange("p two (b s) -> p two b s", s=BW)
            d4 = dsc[:].rearrange("p two (b s) -> p two b s", s=BW)
            r4 = rsc[:].rearrange("p two (b s) -> p two b s", s=BW)
            for c0, c1 in hv:
                xb = dual_band(xt, m - 1, LEV - m, c0, c1)
                nxb = dual_band(nx, m - 1, LEV - m, c0, c1)
                if m == 2:
                    srcb = dual_band(ps1, 0, N + HALF, c0, c1)
                else:
                    s4 = pcq[m - 1][:].rearrange(
                        "p two (b s) -> p two b s", s=BW)
                    srcb = s4[:, :, c0:c1, :]
                tb = t4[:, :, c0:c1, :]
                if fl == "nat":
                    nc.vector.tensor_tensor(tb, srcb, xb, op=MAX)
                else:
                    db = d4[:, :, c0:c1, :]
                    nc.gpsimd.tensor_tensor(db, srcb, nxb, op=ADD)
                    if fl == "pa":
                        rb = r4[:, :, c0:c1, :]
                        nc.scalar.activation(rb, db, AF.Relu)
                        nc.gpsimd.tensor_tensor(tb, xb, rb, op=ADD)
                    else:  # pp: relu on Pool via tensor_scalar
                        nc.gpsimd.tensor_scalar(db, db, 0.0, None, op0=MAX)
                        nc.gpsimd.tensor_tensor(tb, xb, db, op=ADD)
                nc.vector.tensor_tensor_scan(
                    pcq[m][:, 0, c0 * BW:c1 * BW],
                    mC[:, c0 * BW:c1 * BW],
                    tsc[:, 0, c0 * BW:c1 * BW],
                    GUARD, op0=ADD, op1=MIN)
                nc.vector.tensor_tensor_scan(
                    pcq[m][:, 1, c0 * BW:c1 * BW][:, ::-1],
                    mC[:, c0 * BW:c1 * BW],
                    tsc[:, 1, c0 * BW:c1 * BW][:, ::-1],
                    GUARD, op0=ADD, op1=MIN)
            if m == 8:
                merge_layer(8)
            elif m >= 9:
                merge_layer(m)
                if LEV - m >= 1:
                    merge_layer(LEV - m)
        # layer 0: cm[o in 16..30] min= pre_16[o-1] (block i+1)
        nc.vector.tensor_tensor(cm3[:, 0:ic, 16:31], cm3[:, 0:ic, 16:31],
                                plane(16, 0)[:, 1:1 + ic, 0:15], op=MIN)
        return cmin

    with tile.TileContext(nc) as tc:
        with ExitStack() as ctx:
            cpool = ctx.enter_context(tc.tile_pool(name="const", bufs=1))
            inpool = ctx.enter_context(tc.tile_pool(name="in", bufs=2))
            levpool = ctx.enter_context(tc.tile_pool(name="lev", bufs=1))
            cmpool = ctx.enter_context(tc.tile_pool(name="cm", bufs=2))
            pcpool = ctx.enter_context(tc.tile_pool(name="pc", bufs=1))
            sfpool = ctx.enter_context(tc.tile_pool(name="sf", bufs=2))
            ppool = ctx.enter_context(tc.tile_pool(name="ps", bufs=2,
                                                   space="PSUM"))

            NBMAX = max(NHB, NPB)
            CNMAX = (NBMAX // K) * BW
            maskF = cpool.tile([P, NBMAX], f16)
            nc.vector.memset(maskF[:], 0.0)
            mF3 = maskF[:].rearrange("p (b k) -> p b k", k=K)
            nc.vector.memset(mF3[:, :, 0:1], MASKV)
            maskC = cpool.tile([P, CNMAX], f16)
            nc.vector.memset(maskC[:], 0.0)
            mC3 = maskC[:].rearrange("p (b s) -> p b s", s=BW)
            nc.vector.memset(mC3[:, :, 0:1], MASKV)
            ident = cpool.tile([P, P], f16)
            nc.sync.dma_start(ident[:], ID[:])

            pools = (levpool, cmpool, maskF, maskC)

            # ---- phase A: perc (median along H), 4 col-group batches
            pcm = []
            for a in range(4):
                xp = inpool.tile([P, NP], f16, tag="xp", name="xp")
                nc.sync.dma_start(
                    xp[:], XP[:].rearrange("(a p) n -> a p n", p=P)[a])
                cmin = median_banded(pools, xp, xp[:], NP, a)
                pc = pcpool.tile([P, NP], f16, tag=f"pcm{a}", name=f"pcm{a}")
                nc.scalar.copy(pc[:], cmin[:])
                pcm.append(pc)
                nc.sync.dma_start(
                    PM1024[:].rearrange("(a p) o -> a p o", p=P)[a],
                    pc[:, 1024:1025])

            # ---- phase B: harm (median along W) + softmask
            for bi in range(4):
                xh = inpool.tile([P, QB, NH], f16, tag="xh", name="xh")
                nc.sync.dma_start(
                    xh[:], XH[:].rearrange("(b q p) n -> b p q n", p=P, q=QB)[bi])
                cmin = median_banded(pools, xh,
                                     xh[:].rearrange("p q n -> p (q n)"),
                                     NHB, 4 + bi)

                percT = sfpool.tile([P, QB, 512], f16, tag="percT", name="percT")
                for qq in range(QB):
                    r0 = bi * QB * P + qq * P
                    for cg in range(4):
                        ps = ppool.tile([P, P], f16, tag="ps", name="ps")
                        nc.tensor.transpose(
                            ps[:], pcm[cg][:, r0:r0 + P], ident[:])
                        nc.scalar.copy(percT[:, qq, cg * P:(cg + 1) * P], ps[:])

                cm4 = cmin[:].rearrange("p (q n) -> p q n", n=NH)
                h = cm4[:, :, 0:512]
                s_in = xh[:, :, HALF:HALF + 512]
                h2 = sfpool.tile([P, QB, 512], f16, tag="h2", name="h2")
                den = sfpool.tile([P, QB, 512], f16, tag="den", name="den")
                nc.scalar.activation(h2[:], h, AF.Square, scale=64.0)
                nc.scalar.activation(percT[:], percT[:], AF.Square, scale=64.0)
                nc.gpsimd.tensor_tensor(den[:], h2[:], percT[:], op=ADD)
                with nc.allow_low_precision(reason="den >= 1.1e-4 on data"):
                    nc.vector.reciprocal(den[:], den[:])
                    nc.gpsimd.tensor_tensor(h2[:], h2[:], den[:], op=MULT)
                    nc.gpsimd.tensor_tensor(h2[:], h2[:], s_in, op=MULT)
                oh_d = OH[:].rearrange("(b q p) n -> b p q n", p=P, q=QB)[bi]
                nc.sync.dma_start(oh_d, h2[:])

        ret = tc.schedule_and_allocate()
        global _SIM_NS
        try:
            _SIM_NS = ret[1].time
        except Exception:
            _SIM_NS = None

    nc.finalize()
    return nc


def _get_program():
    global _PROGRAM
    if _PROGRAM is None:
        _PROGRAM = _build_program()
    return _PROGRAM


def _host_prep(S):
    ident = np.eye(P, dtype=np.float16)
    S16 = S.astype(np.float16)
    in_maps = []
    for c in range(8):
        pl, h = c >> 1, c & 1
        b, ch = pl >> 1, pl & 1
        Sp = S16[b, ch]
        xh = np.zeros((1024, NH), np.float16)
        lo = 512 * h - HALF
        s0, s1 = max(0, lo), min(1024, lo + NH)
        xh[:, s0 - lo:s1 - lo] = Sp[0:1024, s0:s1]
        xp = np.zeros((512, NP), np.float16)
        xp[:, HALF:HALF + 1025] = Sp[:, 512 * h:512 * h + 512].T
        in_maps.append({"XH": xh, "XP": xp, "ID": ident})
    return in_maps


def _median31_rows(rows):
    R, W = rows.shape
    p = np.pad(rows, ((0, 0), (HALF, HALF)))
    win = np.lib.stride_tricks.sliding_window_view(p, K, axis=1)
    return np.median(win, axis=2).astype(np.float32)


def kernel(S):
    from concourse.bass_utils import run_bass_kernel_spmd

    S = np.asarray(S, np.float32)
    nc = _get_program()
    in_maps = _host_prep(S)
    res = run_bass_kernel_spmd(nc, in_maps, list(range(8)))

    out_h = np.empty_like(S)
    perc_1024 = np.empty((2, 2, 1024), np.float32)
    for c in range(8):
        pl, h = c >> 1, c & 1
        b, ch = pl >> 1, pl & 1
        r = res.results[c]
        out_h[b, ch, 0:1024, 512 * h:512 * h + 512] = r["OH"].astype(np.float32)
        perc_1024[b, ch, 512 * h:512 * h + 512] = \
            r["PM1024"][:, 0].astype(np.float32)
    rows = S[:, :, 1024, :].reshape(4, 1024)
    harm_1024 = _median31_rows(rows).reshape(2, 2, 1024)
    h2 = harm_1024 * harm_1024
    p2 = perc_1024 * perc_1024
    out_h[:, :, 1024, :] = S[:, :, 1024, :] * h2 / (h2 + p2)
    out_p = S - out_h
    return out_h, out_p
